# revision 19
# baseline (speedup 1.0000x reference)
"""Multi-headed attention (B=8, S=1024, D=1024, H=16) on 8 TRN2 NeuronCores.

Strategy: pure data parallelism over the batch — core b computes batch element b
end-to-end (no collectives). All matmuls in bf16 (fp32 PSUM accumulation).

v2 notes (vs the first working version):
  - bk is dropped entirely: adding bk shifts every score of a given query q by
    the same constant (bk . q), and softmax is shift-invariant in k.
  - bv is folded on the host into bo' = bo + Wo @ bv (sum_k p = 1 after the
    softmax, so the V bias contributes a constant row through the O proj).
  - DMA order is criticality-driven: keyT/wk/qT tiles for head-pair 0 first so
    the first exp lands ~7us into the kernel (it was ~45us); V inputs next; the
    rest just-in-time.  ACT (exp) is the scarce engine: 128 ACTIVATEs x ~1.15us
    = ~147us of unavoidable serial exp work; the whole schedule is built to
    keep it fed from ~7us onward.
  - V projection moved out of the prologue into early pipeline steps (its
    inputs only arrive ~15-20us in); p@v for a pair runs two steps after its
    scores (was one), with pT buffered at half-pair granularity (bufs=6) so
    the late V tiles don't stall the exp stream.

Per-core dataflow (everything "T" is feature-major [D, S]):
  inputs (host-pretransposed, bf16): qT, keyT, valT, wkT, wvT, woT
  1. K_T[d_out, s]  = matmul(lhsT=wkT, rhs=keyT)           (no bias; see above)
  2. V[s, d_out]    = matmul(lhsT=valT, rhs=wvT) -> packed [s, h, 65]
                      with a ones column per head (gives softmax denominators
                      for free inside the p@v matmul)
  3. per head h: scoresT[k, q] = matmul(lhsT=K_T_h[64,128], rhs=qT_h[64,512])
                 pT = exp(scoresT / 8)  (ACT; max-subtraction skipped — scores
                 are provably small for this problem)
  4. xT_h[65, q] accum = matmul(lhsT=[V_h | 1][128,65], rhs=pT[128,512]);
     row 64 = softmax denominator; normalize rows 0..63 by its reciprocal
     (partition-broadcast via DMA)
  5. O[s, d_out] = matmul(lhsT=xT, rhs=woT) + bo' -> DMA out (f32)
"""

import numpy as np
import ml_dtypes

import concourse.bass as bass
import concourse.bacc as bacc
import concourse.mybir as mybir
import concourse.tile as tile
from contextlib import ExitStack

B, S, D, H = 8, 1024, 1024, 16
P = 128
DK = D // H          # 64
NCH = D // P         # 8
QC = 512             # free-dim chunk (one PSUM bank)
NQC = S // QC        # 2
SCALE = 1.0 / float(np.sqrt(DK))  # 0.125
N_CORES = 8

BF16 = mybir.dt.bfloat16
F32 = mybir.dt.float32
ADD = mybir.AluOpType.add
MULT = mybir.AluOpType.mult
EXP = mybir.ActivationFunctionType.Exp

_CACHE = {}


def _build_nc():
    nc = bacc.Bacc(None)

    qT_d = nc.dram_tensor("qT", [NCH, P, S], BF16, kind="ExternalInput")
    keyT_d = nc.dram_tensor("keyT", [NCH, P, S], BF16, kind="ExternalInput")
    valT_d = nc.dram_tensor("valT", [NCH, P, S], BF16, kind="ExternalInput")
    wkT_d = nc.dram_tensor("wkT", [NCH, P, D], BF16, kind="ExternalInput")
    wvT_d = nc.dram_tensor("wvT", [NCH, P, D], BF16, kind="ExternalInput")
    woT_d = nc.dram_tensor("woT", [NCH, P, D], BF16, kind="ExternalInput")
    bo_d = nc.dram_tensor("bo", [D], F32, kind="ExternalInput")
    out_d = nc.dram_tensor("out", [S, D], F32, kind="ExternalOutput")

    with tile.TileContext(nc) as tc:
        with ExitStack() as ctx:
            const = ctx.enter_context(tc.tile_pool(name="const", bufs=1))
            big = ctx.enter_context(tc.tile_pool(name="big", bufs=1))
            wpool = ctx.enter_context(tc.tile_pool(name="wpool", bufs=1))
            ppool = ctx.enter_context(tc.tile_pool(name="ppool", bufs=2))
            opool = ctx.enter_context(tc.tile_pool(name="opool", bufs=2))
            rpool = ctx.enter_context(tc.tile_pool(name="rpool", bufs=2))
            xpool = ctx.enter_context(tc.tile_pool(name="xpool", bufs=9))
            proj_ps = ctx.enter_context(
                tc.tile_pool(name="proj_ps", bufs=2, space="PSUM")
            )
            sc_ps = ctx.enter_context(tc.tile_pool(name="sc_ps", bufs=2, space="PSUM"))
            xt_ps = ctx.enter_context(tc.tile_pool(name="xt_ps", bufs=2, space="PSUM"))

            # --- SBUF resident tensors ---
            qT = big.tile([P, NCH, S], BF16, tag="qT")
            keyT = big.tile([P, NCH, S], BF16, tag="share1")  # reused later by xT
            # valT in two s-halves; the 6-slot/8KB "ptv" tag later hosts the
            # half-pair pT tiles (valT is dead after the V projection)
            valT_a = big.tile([P, NCH, QC], BF16, tag="ptv", bufs=6)
            valT_b = big.tile([P, NCH, QC], BF16, tag="ptv", bufs=6)
            kT = big.tile([P, NCH, S], BF16, tag="kT")
            vpad = big.tile([P, NCH, H, DK + 1], BF16, tag="vpad")
            wk = wpool.tile([P, NCH, D], BF16, tag="wk")
            wv = wpool.tile([P, NCH, D], BF16, tag="wv")
            wo = wpool.tile([P, NCH, D], BF16, tag="wo")
            bo_b = const.tile([P, D], F32, tag="bo")

            # --- input DMAs, criticality order ---
            # (keyT+wk+qT slices for early score/exp start; V inputs next; the
            #  rest just-in-time.  ~353 GB/s aggregate on this input stream.)
            def ld(dst, src):
                nc.sync.dma_start(out=dst, in_=src)

            r = lambda ap: ap.rearrange("c p f -> p c f")
            ld(keyT[:, :, 0:QC], r(keyT_d[:, :, 0:QC]))
            ld(wk[:, :, 0:P], r(wkT_d[:, :, 0:P]))
            ld(qT[:, 0, :], qT_d[0])
            ld(keyT[:, :, QC:S], r(keyT_d[:, :, QC:S]))
            ld(wk[:, :, P : 2 * P], r(wkT_d[:, :, P : 2 * P]))
            ld(qT[:, 1, :], qT_d[1])
            ld(valT_a[:], r(valT_d[:, :, 0:QC]))
            ld(valT_b[:], r(valT_d[:, :, QC:S]))
            ld(wv[:, :, 0:QC], r(wvT_d[:, :, 0:QC]))
            ld(wv[:, :, QC:D], r(wvT_d[:, :, QC:D]))
            ld(wk[:, :, 2 * P : 4 * P], r(wkT_d[:, :, 2 * P : 4 * P]))
            ld(qT[:, 2, :], qT_d[2])
            ld(wk[:, :, 4 * P : 8 * P], r(wkT_d[:, :, 4 * P : 8 * P]))
            ld(qT[:, 3, :], qT_d[3])
            ld(qT[:, 4:8, :], r(qT_d[4:8]))
            ld(wo[:], r(woT_d[:]))
            ld(bo_b[:], bo_d[:][None, :].to_broadcast((P, D)))

            # --- 1. K_T = Wk @ key.T  (feature-major) ---
            # m-tile 0 runs in the prologue, j-split so scores can start after
            # only half of keyT has landed; tiles 1..7 are step fillers.
            ktp_ps = {}

            def emit_ktproj_j(m, j):
                """K-proj m-tile, one q-half j: 8 contraction MMs + evict."""
                ps = proj_ps.tile([P, QC], F32, tag="pp", name=f"kp{m}_{j}")
                for c in range(NCH):
                    nc.tensor.matmul(
                        ps[:],
                        wk[:, c, m * P : (m + 1) * P],
                        keyT[:, c, j * QC : (j + 1) * QC],
                        start=(c == 0),
                        stop=(c == NCH - 1),
                    )
                nc.vector.tensor_copy(kT[:, m, j * QC : (j + 1) * QC], ps[:])

            def emit_ktproj(m, half):
                emit_ktproj_j(m, half)

            # --- 2. V = value @ Wv.T (token-major, head-padded w/ ones) ---
            vp_ps = {}

            def emit_vproj(t, half):
                if half == 0:
                    vp_ps[t] = [
                        proj_ps.tile([P, QC], F32, tag="pp", name=f"vp{t}_{j}")
                        for j in range(NQC)
                    ]
                ps = vp_ps[t]
                vhalf = valT_a if t < 4 else valT_b
                j = half
                for c in range(NCH):
                    nc.tensor.matmul(
                        ps[j][:],
                        vhalf[:, c, (t % 4) * P : (t % 4 + 1) * P],
                        wv[:, c, j * QC : (j + 1) * QC],
                        start=(c == 0),
                        stop=(c == NCH - 1),
                    )
                if half == 0:
                    return
                hpc = QC // DK  # heads per psum chunk
                for j in range(NQC):
                    nc.vector.tensor_copy(
                        vpad[:, t, j * hpc : (j + 1) * hpc, 0:DK],
                        ps[j][:].rearrange("p (h d) -> p h d", d=DK),
                    )
                nc.vector.memset(vpad[:, t, :, DK : DK + 1], 1.0)

            # --- 3+4. per-head attention, q-chunk-outer, head-pipelined ---
            xT = big.tile([P, NCH, S], BF16, tag="share1")  # reuses keyT slot

            def emit_sc_kt(m, j, kt, pt_lo, pt_hi):
                """scoresT k-tile kt for BOTH heads of pair m (q-chunk j).

                The two heads sit at partition offsets 0/64 -> their K=64
                matmuls land on row-tiles (0,0)/(64,0); emitted adjacently
                they stream through the PE array concurrently. One 2-bank
                psum holds both (bank per head); one exp evicts both."""
                sp = sc_ps.tile([P, 2, QC], F32, tag="sp", name=f"sp{m}{j}{kt}")
                for odd in range(2):
                    off = odd * DK
                    nc.tensor.matmul(
                        sp[:, odd, :],
                        kT[off : off + DK, m, kt * P : (kt + 1) * P],
                        qT[off : off + DK, m, j * QC : (j + 1) * QC],
                        start=True,
                        stop=True,
                    )
                pt = pt_lo if kt < 4 else pt_hi
                nc.scalar.activation(pt[:, kt % 4, :, :], sp[:], EXP, scale=SCALE)

            xp_map = {}

            def emit_pv_mms(p, pt_lo, pt_hi, kcs):
                """p@v accumulation matmuls for position p over k-chunks kcs."""
                j, h = divmod(p, H)
                if p not in xp_map:
                    xp_map[p] = xt_ps.tile([DK + 1, QC], F32, tag="xp", name=f"xp{p}")
                xp = xp_map[p]
                for kc in kcs:
                    pt = pt_lo if kc < 4 else pt_hi
                    nc.tensor.matmul(
                        xp[:],
                        vpad[:, kc, h, :],
                        pt[:, kc % 4, h % 2, :],
                        start=(kc == 0),
                        stop=(kc == NCH - 1),
                    )

            def emit_pv_fin(p, dcols, xsb_map):
                """evict unnormalized x -> SBUF, denom row -> batch collector.

                Engine APs need 32-aligned start partitions, so the 4 denoms
                of a batch land at partitions 0/32/64/96 of one collector."""
                xp = xp_map.pop(p)
                xsb = xpool.tile([DK, QC], BF16, tag="xsb", name=f"xsb{p}")
                nc.vector.tensor_copy(xsb[:], xp[0:DK, :])
                b, r = divmod(p, 4)
                if p >= 30:  # last two heads: own 2-head batch (shorter tail chain)
                    b, r = 8, p - 30
                if r == 0:
                    dcols[b] = rpool.tile([97, QC], F32, tag="dcol", name=f"dc{b}")
                    nc.vector.memset(dcols[b][:], 1.0)  # only rows 0/32/64/96 matter
                nc.vector.tensor_copy(dcols[b][32 * r : 32 * r + 1, :], xp[DK : DK + 1, :])
                xsb_map[p] = xsb

            def emit_recip_half(j, pb, half, dcols, rcols):
                """half of the batch reciprocal (split so the DVE queue never
                blocks >~1.7us in front of the xp-slot-freeing copies)."""
                if half == 0:
                    rcols[pb // 4] = rpool.tile([97, QC], F32, tag="rcol", name=f"rc{pb}")
                sl = slice(half * (QC // 2), (half + 1) * (QC // 2))
                nc.vector.reciprocal(rcols[pb // 4][:, sl], dcols[pb // 4][:, sl])

            def emit_bcast(j, p, rcols, rb_map):
                r = 32 * (p % 4)
                # partition_broadcast ucode reads via Q7 core 0 -> input
                # must sit at base partition 0; bounce the row there.
                rb0 = rpool.tile([1, QC], F32, tag="rb0", name=f"rb0_{p}")
                nc.vector.tensor_copy(rb0[:], rcols[p // 4][r : r + 1, :])
                rb = rpool.tile([DK, QC], F32, tag="rb", name=f"rb{p}", bufs=6)
                nc.gpsimd.partition_broadcast(rb[:], rb0[:])
                rb_map[p] = rb

            def emit_norm_mult(j, p, xsb_map, rb_map):
                """deferred multiply (by now the broadcast is long done)."""
                ch, off = divmod(p % H, 2)
                off *= DK
                nc.vector.tensor_tensor(
                    xT[off : off + DK, ch, j * QC : (j + 1) * QC],
                    xsb_map[p][:],
                    rb_map[p][:],
                    op=MULT,
                )

            op_ps = {}

            def emit_oproj(t, half=None):
                """O = x @ Wo.T + bo' for s-tile t (needs all of xT cols of t)."""
                if half in (0, None):
                    op_ps[t] = [
                        proj_ps.tile([P, QC], F32, tag="pp", name=f"op{t}_{j}")
                        for j in range(NQC)
                    ]
                ps = op_ps[t]
                cs = range(NCH) if half is None else range(half * 4, half * 4 + 4)
                for c in cs:
                    st = xT[:, c, t * P : (t + 1) * P]
                    for j in range(NQC):
                        nc.tensor.matmul(
                            ps[j][:],
                            st,
                            wo[:, c, j * QC : (j + 1) * QC],
                            start=(c == 0),
                            stop=(c == NCH - 1),
                        )
                if half == 0:
                    return
                for j in range(NQC):
                    ot = opool.tile([P, QC], F32, tag="ot", name=f"ot{t}_{j}")
                    nc.vector.tensor_tensor(
                        ot[:], ps[j][:], bo_b[:, j * QC : (j + 1) * QC], op=ADD
                    )
                    nc.sync.dma_start(
                        out=out_d[t * P : (t + 1) * P, j * QC : (j + 1) * QC],
                        in_=ot[:],
                    )

            # ---- flat 32-step pipeline over (chunk, head) ----
            # position p = 16*j + h. Norm chain of each 4-head batch is
            # spread one small op-group per later step (crossing chunk
            # boundaries) so no engine queue ever blocks the PE for long.
            dcols = {}
            xsb_map = {}
            rcols = {}
            rb_map = {}
            NPOS = NQC * H

            def norm_step(s):
                for p, acts in (
                    (s - 4, "r0"),
                    (s - 5, "r1"),
                    (s - 6, "b01"),
                    (s - 7, "b23m0"),
                    (s - 8, "m12"),
                    (s - 9, "m3"),
                ):
                    if p < 0 or p % 4 != 0 or p >= NPOS or p == 28:
                        continue
                    j = p // H
                    if acts == "r0":
                        emit_recip_half(j, p, 0, dcols, rcols)
                    elif acts == "r1":
                        emit_recip_half(j, p, 1, dcols, rcols)
                    elif acts == "b01":
                        emit_bcast(j, p, rcols, rb_map)
                        emit_bcast(j, p + 1, rcols, rb_map)
                    elif acts == "b23m0":
                        emit_bcast(j, p + 2, rcols, rb_map)
                        emit_bcast(j, p + 3, rcols, rb_map)
                        emit_norm_mult(j, p, xsb_map, rb_map)
                    elif acts == "m12":
                        emit_norm_mult(j, p + 1, xsb_map, rb_map)
                        emit_norm_mult(j, p + 2, xsb_map, rb_map)
                    else:
                        emit_norm_mult(j, p + 3, xsb_map, rb_map)

            # prologue: K-proj m0 (j-split), nothing else — everything other
            # than the K path is still in flight on the DMA rings.
            emit_ktproj_j(0, 0)
            emit_ktproj_j(0, 1)

            # per-step fillers: K-proj tile m+1 during step m (chunk 0);
            # V-proj halves spread over steps 0-4 (inputs land ~15-20us);
            # O-proj s-tiles 0-2 during chunk-1 steps (tile 3+ in the tail).
            step_fillers = {
                0: [(emit_ktproj, 1, 0), (emit_ktproj, 1, 1),
                    (emit_vproj, 0, 0), (emit_vproj, 0, 1)],
                1: [(emit_ktproj, 2, 0), (emit_ktproj, 2, 1),
                    (emit_vproj, 1, 0), (emit_vproj, 1, 1),
                    (emit_vproj, 2, 0), (emit_vproj, 2, 1),
                    (emit_vproj, 3, 0), (emit_vproj, 3, 1)],
                2: [(emit_ktproj, 3, 0), (emit_ktproj, 3, 1),
                    (emit_vproj, 4, 0), (emit_vproj, 4, 1),
                    (emit_vproj, 5, 0), (emit_vproj, 5, 1)],
                3: [(emit_ktproj, 4, 0), (emit_ktproj, 4, 1),
                    (emit_vproj, 6, 0), (emit_vproj, 6, 1),
                    (emit_vproj, 7, 0), (emit_vproj, 7, 1)],
                4: [(emit_ktproj, 5, 0), (emit_ktproj, 5, 1)],
                5: [(emit_ktproj, 6, 0), (emit_ktproj, 6, 1),
                    (emit_ktproj, 7, 0), (emit_ktproj, 7, 1)],
                13: [(emit_oproj, 0, 0), (emit_oproj, 0, 1)],
                14: [(emit_oproj, 1, 0), (emit_oproj, 1, 1)],
                15: [(emit_oproj, 2, 0), (emit_oproj, 2, 1)],
            }

            # pair-step loop: heads 2m/2m+1 of q-chunk j scored+exp'd in step
            # ps_; their p@v runs in step ps_+2 (half-pair pT buffering gives
            # the V projection time to land without stalling the exp stream).
            pt_map = {}
            KC_A = tuple(range(NCH // 2))
            KC_B = tuple(range(NCH // 2, NCH))
            for ps_ in range(NPOS // 2):
                j, m = divmod(ps_, H // 2)
                fillers = step_fillers.get(ps_, [])

                def filler(i):
                    if i < len(fillers):
                        f, a, b = fillers[i]
                        f(a, b)

                pt_lo = big.tile(
                    [P, NCH // 2, 2, QC], BF16, tag="ptv", bufs=6, name=f"ptl{ps_}"
                )
                pt_hi = big.tile(
                    [P, NCH // 2, 2, QC], BF16, tag="ptv", bufs=6, name=f"pth{ps_}"
                )
                if ps_ == 2:
                    # pair 0's KC_A only: the V tiles feeding KC_B (6,7) are
                    # still being projected; KC_B must follow them in program
                    # order or the in-order PE queue would deadlock.
                    pp = pt_map[0]
                elif ps_ == 3:
                    pp = None  # handled in the post-filler block below
                else:
                    pp = pt_map.pop(ps_ - 2, None)  # (lo, hi) of pair to p@v
                d0 = 2 * ps_ - 4
                d1 = 2 * ps_ - 3
                # ACT feed first: two score k-tiles before any filler work
                emit_sc_kt(m, j, 0, pt_lo, pt_hi)
                emit_sc_kt(m, j, 1, pt_lo, pt_hi)
                if ps_ == 4:
                    # finish pair 1 before pair 2's p@v below needs xp slots
                    p1 = pt_map.pop(1)
                    emit_pv_mms(2, p1[0], p1[1], KC_B)
                    emit_pv_fin(2, dcols, xsb_map)
                    emit_pv_mms(3, p1[0], p1[1], KC_B)
                    emit_pv_fin(3, dcols, xsb_map)
                filler(0)
                emit_sc_kt(m, j, 2, pt_lo, pt_hi)
                filler(1)
                if ps_ == 2:
                    emit_pv_mms(0, pp[0], pp[1], KC_A)
                elif pp is not None:
                    emit_pv_mms(d0, pp[0], pp[1], KC_A)
                emit_sc_kt(m, j, 3, pt_lo, pt_hi)
                if pp is not None and ps_ != 2:
                    emit_pv_mms(d0, pp[0], pp[1], KC_B)
                    emit_pv_fin(d0, dcols, xsb_map)
                filler(2)
                emit_sc_kt(m, j, 4, pt_lo, pt_hi)
                if ps_ == 2:
                    emit_pv_mms(1, pp[0], pp[1], KC_A)
                elif pp is not None:
                    emit_pv_mms(d1, pp[0], pp[1], KC_A)
                emit_sc_kt(m, j, 5, pt_lo, pt_hi)
                if pp is not None and ps_ != 2:
                    emit_pv_mms(d1, pp[0], pp[1], KC_B)
                    emit_pv_fin(d1, dcols, xsb_map)
                filler(3)
                emit_sc_kt(m, j, 6, pt_lo, pt_hi)
                emit_sc_kt(m, j, 7, pt_lo, pt_hi)
                for i in range(4, len(fillers)):
                    filler(i)
                if ps_ == 3:
                    # all vpad tiles exist now: finish pair 0, start pair 1
                    # (spread the catch-up over steps 3-4 so no single step
                    #  starves the exp stream behind a p@v burst)
                    p0 = pt_map.pop(0)
                    emit_pv_mms(0, p0[0], p0[1], KC_B)
                    emit_pv_fin(0, dcols, xsb_map)
                    emit_pv_mms(1, p0[0], p0[1], KC_B)
                    emit_pv_fin(1, dcols, xsb_map)
                    p1 = pt_map[1]
                    emit_pv_mms(2, p1[0], p1[1], KC_A)
                    emit_pv_mms(3, p1[0], p1[1], KC_A)
                pt_map[ps_] = (pt_lo, pt_hi)
                norm_step(2 * ps_ - 4)
                norm_step(2 * ps_ - 3)
            # ---- tail: pairs 14 (pos 28-29, batch 7) and 15 (pos 30-31,
            # batch 8) still need p@v; interleave their short norm chains
            # with the remaining O-projection tiles.
            pp14 = pt_map.pop(NPOS // 2 - 2)
            pp15 = pt_map.pop(NPOS // 2 - 1)
            # finish batch-6 norm chains now (their fins landed in step 15)
            for s in range(NPOS - 4, NPOS + 2):
                norm_step(s)

            # O-proj tiles 3-5 start early: di-chunks 0-5 (heads 0-13 of
            # q-chunk 1) are normalized long before the last heads; only
            # chunks 6-7 wait on the tail norm chains.  Tiles 3 and 5 borrow
            # freed score-psum slots so the proj pool stays with tile 4.
            sp3 = sc_ps.tile([P, 2, QC], F32, tag="sp", name="op3ps")
            sp5 = sc_ps.tile([P, 2, QC], F32, tag="sp", name="op5ps")
            op3ap = [sp3[:, j, :] for j in range(NQC)]
            op5ap = [sp5[:, j, :] for j in range(NQC)]
            op4ap = [
                proj_ps.tile([P, QC], F32, tag="pp", name=f"op4_{j}")[:]
                for j in range(NQC)
            ]

            def emit_oproj_cs(t, ps, cs):
                for c in cs:
                    st = xT[:, c, t * P : (t + 1) * P]
                    for j in range(NQC):
                        nc.tensor.matmul(
                            ps[j],
                            st,
                            wo[:, c, j * QC : (j + 1) * QC],
                            start=(c == 0),
                            stop=(c == NCH - 1),
                        )

            def emit_oproj_evict(t, ps):
                for j in range(NQC):
                    ot = opool.tile([P, QC], F32, tag="ot", name=f"ot{t}_{j}")
                    nc.vector.tensor_tensor(
                        ot[:], ps[j], bo_b[:, j * QC : (j + 1) * QC], op=ADD
                    )
                    nc.sync.dma_start(
                        out=out_d[t * P : (t + 1) * P, j * QC : (j + 1) * QC],
                        in_=ot[:],
                    )

            emit_oproj_cs(3, op3ap, range(NCH))  # q-chunk 0: fully ready
            emit_oproj_evict(3, op3ap)
            emit_oproj_cs(4, op4ap, range(6))
            emit_pv_mms(NPOS - 4, pp14[0], pp14[1], KC_A + KC_B)
            emit_pv_fin(NPOS - 4, dcols, xsb_map)
            emit_oproj_cs(5, op5ap, range(6))
            emit_pv_mms(NPOS - 3, pp14[0], pp14[1], KC_A + KC_B)
            emit_pv_fin(NPOS - 3, dcols, xsb_map)
            emit_recip_half(1, 28, 0, dcols, rcols)
            emit_pv_mms(NPOS - 2, pp15[0], pp15[1], KC_A + KC_B)
            emit_recip_half(1, 28, 1, dcols, rcols)
            emit_pv_fin(NPOS - 2, dcols, xsb_map)
            emit_bcast(1, 28, rcols, rb_map)
            emit_bcast(1, 29, rcols, rb_map)
            emit_pv_mms(NPOS - 1, pp15[0], pp15[1], KC_A + KC_B)
            emit_norm_mult(1, 28, xsb_map, rb_map)
            emit_norm_mult(1, 29, xsb_map, rb_map)
            emit_pv_fin(NPOS - 1, dcols, xsb_map)
            emit_oproj_cs(4, op4ap, [6])
            emit_oproj_cs(5, op5ap, [6])
            # batch 8 chain (rows 0/32 of dcols[8])
            rcol8 = rpool.tile([33, QC], F32, tag="rcol", name="rc8")
            nc.vector.reciprocal(rcol8[:, 0 : QC // 2], dcols[8][0:33, 0 : QC // 2])
            nc.vector.reciprocal(rcol8[:, QC // 2 : QC], dcols[8][0:33, QC // 2 : QC])
            for i, pf in enumerate((30, 31)):
                rb0f = rpool.tile([1, QC], F32, tag="rb0", name=f"rb0f{pf}")
                nc.vector.tensor_copy(rb0f[:], rcol8[32 * i : 32 * i + 1, :])
                rbf = rpool.tile([DK, QC], F32, tag="rb", name=f"rbf{pf}", bufs=6)
                nc.gpsimd.partition_broadcast(rbf[:], rb0f[:])
                ch, off = divmod(pf % H, 2)
                off *= DK
                nc.vector.tensor_tensor(
                    xT[off : off + DK, ch, QC : 2 * QC],
                    xsb_map[pf][:],
                    rbf[:],
                    op=MULT,
                )
            emit_oproj_cs(4, op4ap, [7])
            emit_oproj_evict(4, op4ap)
            emit_oproj_cs(5, op5ap, [7])
            emit_oproj_evict(5, op5ap)
            # last two s-tiles the classic way (proj psum freed by tile 4)
            emit_oproj(6)
            emit_oproj(7)

    nc.finalize()
    return nc


def get_nc():
    if "nc" not in _CACHE:
        _CACHE["nc"] = _build_nc()
    return _CACHE["nc"]


def _tp_bf16(a):
    """[X, Y] f32 -> transposed bf16 [NCH, P, Y]."""
    return (
        np.ascontiguousarray(np.asarray(a, dtype=np.float32).T)
        .astype(ml_dtypes.bfloat16)
        .reshape(NCH, P, -1)
    )


def make_in_maps(query, key, value, Wk, bk, Wv, bv, Wo, bo):
    # bk shifts all scores of a query equally -> cancelled by softmax: dropped.
    # bv contributes Wo @ bv to every output row (sum_k p == 1): fold into bo.
    Wo = np.asarray(Wo, dtype=np.float32)
    bo_eff = (
        np.asarray(bo, dtype=np.float32)
        + Wo @ np.asarray(bv, dtype=np.float32)
    ).astype(np.float32)
    wkT = _tp_bf16(Wk)
    wvT = _tp_bf16(Wv)
    woT = _tp_bf16(Wo)
    in_maps = []
    for b in range(B):
        in_maps.append(
            {
                "qT": _tp_bf16(query[b]),
                "keyT": _tp_bf16(key[b]),
                "valT": _tp_bf16(value[b]),
                "wkT": wkT,
                "wvT": wvT,
                "woT": woT,
                "bo": bo_eff,
            }
        )
    return in_maps


def run(trace=False, **inputs):
    from concourse.bass_utils import run_bass_kernel_spmd

    nc = get_nc()
    in_maps = make_in_maps(**inputs)
    res = run_bass_kernel_spmd(nc, in_maps, list(range(N_CORES)), trace=trace)
    out = np.stack([res.results[i]["out"] for i in range(N_CORES)], axis=0)
    return out, res


def kernel(**inputs):
    out, _ = run(trace=False, **inputs)
    return out
